# revision 10
# baseline (speedup 1.0000x reference)
"""GNN message-passing kernel for trn2 (8 NeuronCores, SPMD).

Sharding: nodes split evenly across cores; each edge owned by its receiver's
core. Within a core, nodes are bin-packed into groups of 128 slots with
balanced edge counts; each group's edge list is padded to a fixed budget B
(multiple of 512). Segment softmax + aggregation are core-local (one-hot
matmul per group; shiftless exp is numerically safe for this model's score
range). Node-update MLP is data-parallel; results are scattered into a
per-core staging buffer and AllGathered into the replicated x.

Layouts: activations flow feature-major [feat(part), item(free)] through all
MLP matmuls (lhsT = weight [K=din, M=dout], rhs = activation). Gathered rows
arrive item-major and are PE-transposed. Matmul operands are bf16 (FWL fast
weight loads, full-rate PE); PSUM accumulation, biases and the softmax are
fp32.
"""
import sys, types
import numpy as np
import ml_dtypes

import concourse.bass as bass
import concourse.tile as tile
from concourse import bacc, mybir
from concourse.masks import make_identity

F32 = mybir.dt.float32
BF16 = mybir.dt.bfloat16
I32 = mybir.dt.int32
AF = mybir.ActivationFunctionType
ALU = mybir.AluOpType

P = 128
NP_BF16 = ml_dtypes.bfloat16


def install_ntff_shim():
    """The agent image lacks antenv.axon_hooks; install a shim so trace=True works."""
    if "antenv.axon_hooks" in sys.modules:
        return
    import antenv
    _hooks = types.ModuleType("antenv.axon_hooks")
    _hooks._hook = None
    def _set(h):
        _hooks._hook = h
    def _get():
        return _hooks._hook
    _hooks.set_axon_ntff_profile_hook = _set
    _hooks.get_axon_ntff_profile_hook = _get
    sys.modules["antenv.axon_hooks"] = _hooks
    antenv.axon_hooks = _hooks
    try:
        from trn_agent_boot.trn_boot import _ntff_profile_via_ctypes
        _set(_ntff_profile_via_ctypes("/opt/axon/libaxon_pjrt.so"))
    except Exception:
        pass


def ceil_div(a, b):
    return (a + b - 1) // b


# ----------------------------------------------------------------------------
# Host preprocessing
# ----------------------------------------------------------------------------

def preprocess(nodes, edges, senders, receivers, n_cores):
    """Shard + permute. Returns (cfg, per_core_inputs:list[dict])."""
    N, D_NODE = nodes.shape
    E, D_EDGE = edges.shape
    senders = np.asarray(senders).astype(np.int64)
    receivers = np.asarray(receivers).astype(np.int64)
    assert N % n_cores == 0
    npc = N // n_cores                      # nodes per core
    gpc = ceil_div(npc, P)                  # groups per core
    slots = gpc * P

    deg = np.bincount(receivers, minlength=N)
    edges_of_core = [np.where(receivers // npc == c)[0] for c in range(n_cores)]

    # Bin-pack each core's nodes into gpc groups of <=128 slots, balancing edges.
    group_of_node = np.full(N, -1, np.int64)
    slot_of_node = np.full(N, -1, np.int64)
    max_group_edges = 0
    for c in range(n_cores):
        local = np.arange(npc * c, npc * (c + 1))
        order = local[np.argsort(-deg[local], kind="stable")]
        gsum = np.zeros(gpc, np.int64)
        gcnt = np.zeros(gpc, np.int64)
        for n in order:
            cand = np.where(gcnt < P)[0]
            g = cand[np.argmin(gsum[cand])]
            group_of_node[n] = g
            slot_of_node[n] = gcnt[g]
            gsum[g] += deg[n]
            gcnt[g] += 1
        max_group_edges = max(max_group_edges, int(gsum.max()))

    B = max(512, ceil_div(max_group_edges, 512) * 512)   # edge budget per group
    E_pad = gpc * B

    per_core = []
    for c in range(n_cores):
        eix = edges_of_core[c]
        g_of_e = group_of_node[receivers[eix]]
        send_idx = np.zeros(E_pad, np.int64)
        recv_idx = np.zeros(E_pad, np.int64)
        recv_slot = np.full(E_pad, 255.0, np.float32)
        edge_perm = np.full(E_pad, -1, np.int64)
        for g in range(gpc):
            ge = eix[g_of_e == g]
            assert len(ge) <= B, f"group overflow {len(ge)} > {B}"
            base = g * B
            send_idx[base:base + len(ge)] = senders[ge]
            recv_idx[base:base + len(ge)] = receivers[ge]
            recv_slot[base:base + len(ge)] = slot_of_node[receivers[ge]].astype(np.float32)
            edge_perm[base:base + len(ge)] = ge

        edgesT = np.zeros((D_EDGE, E_pad), np.float32)
        real = edge_perm >= 0
        edgesT[:, real] = np.asarray(edges, np.float32)[edge_perm[real]].T

        upd_gather = np.zeros(slots, np.int64)
        scat_idx = np.full(slots, 2_000_000, np.int64)
        local = np.arange(npc * c, npc * (c + 1))
        s_of = group_of_node[local] * P + slot_of_node[local]
        upd_gather[s_of] = local
        scat_idx[s_of] = local - npc * c

        oh2 = np.zeros((P, E_pad), np.float32)
        real_j = np.where(recv_slot < P)[0]
        oh2[recv_slot[real_j].astype(np.int64), real_j] = 1.0

        per_core.append({
            "send_idx": send_idx.astype(np.int32).reshape(-1, P).T.copy(),
            "oh2": oh2.astype(NP_BF16),
            "recv_slot": recv_slot.reshape(-1, P).T.copy(),
            "edgesT": edgesT.astype(NP_BF16),
            "upd_gather": upd_gather.astype(np.int32).reshape(-1, P).T.copy(),
            "scat_idx": scat_idx.astype(np.int32).reshape(-1, P).T.copy(),
        })

    s0 = int(group_of_node[0] * P + slot_of_node[0])
    cfg = dict(N=N, E=E, D_NODE=D_NODE, D_EDGE=D_EDGE, n_cores=n_cores,
               npc=npc, gpc=gpc, slots=slots, B=B, E_pad=E_pad, s0=s0)
    return cfg, per_core


def pack_params(params, n_layers):
    """Flatten params into {name: np.ndarray|float}. Weights are bf16 in
    matmul lhsT layout [din, dout]; biases are f32 [128, mch].

    msg0 weight rows are reordered from [edge, xs, xr] to [xs, xr, edge] so
    that 128-row K-chunks align with the rhs source tiles.
    """
    out = {}

    def put_w(name, w):
        out[name] = np.ascontiguousarray(np.asarray(w, np.float32)).astype(NP_BF16)

    def put_b(name, b):
        b = np.asarray(b, np.float32)
        dout = b.shape[0]
        mch = ceil_div(dout, P)
        bp = np.zeros((mch * P,), np.float32)
        bp[:dout] = b
        out[name] = bp.reshape(mch, P).T.copy()

    def put_mlp(prefix, mlp):
        for i, d in enumerate(mlp):
            put_w(f"{prefix}{i}_w", d["w"])
            put_b(f"{prefix}{i}_b", d["b"])

    for l in range(n_layers):
        p = params[f"layer{l}"]
        put_mlp(f"l{l}_msg", p["msg"])
        w1 = np.asarray(p["msg"][0]["w"], np.float32)
        d_in = (w1.shape[0] - 16) // 2  # D_EDGE = 16 for this model family
        d_e = w1.shape[0] - 2 * d_in
        out[f"l{l}_msg0_w"] = np.ascontiguousarray(
            np.concatenate([w1[d_e:d_e + d_in], w1[d_e + d_in:], w1[:d_e]],
                           axis=0)).astype(NP_BF16)
        put_mlp(f"l{l}_gate", p["gate"])
        put_w(f"l{l}_attn_w", p["attn"]["w"])
        out[f"l{l}_attn_b"] = float(np.asarray(p["attn"]["b"]).reshape(()))
        put_mlp(f"l{l}_upd", p["upd"])
    put_mlp("head", params["head"])
    return out


# ----------------------------------------------------------------------------
# Bass program builder
# ----------------------------------------------------------------------------

class Builder:
    def __init__(self, cfg, pp, n_layers):
        self.cfg = cfg
        self.pp = pp
        self.n_layers = n_layers
        self.MSG = pp["l0_msg2_w"].shape[1]
        self.OUT = pp[f"l{n_layers-1}_upd2_w"].shape[1]
        self.nc = bacc.Bacc("TRN2", target_bir_lowering=False, debug=False,
                            num_devices=cfg["n_cores"])
        self.input_names = []

    def inp(self, name, shape, dtype):
        h = self.nc.dram_tensor(name, list(shape), dtype, kind="ExternalInput")
        self.input_names.append(name)
        return h

    def build(self):
        cfg = self.cfg
        nc = self.nc
        gpc, E_pad = cfg["gpc"], cfg["E_pad"]
        D_EDGE, D_NODE = cfg["D_EDGE"], cfg["D_NODE"]

        nodes_in = self.inp("nodes", [cfg["N"], D_NODE], BF16)
        edgesT_in = self.inp("edgesT", [D_EDGE, E_pad], BF16)
        send_in = self.inp("send_idx", [P, E_pad // P], I32)
        oh2_in = self.inp("oh2", [P, E_pad], BF16)
        rslot_in = self.inp("recv_slot", [P, E_pad // P], F32)
        updg_in = self.inp("upd_gather", [P, gpc], I32)
        scat_in = self.inp("scat_idx", [P, gpc], I32)
        iota_in = self.inp("iota_mat", [P, P], F32)

        w_in = {}
        for name, arr in self.pp.items():
            if isinstance(arr, float):
                continue
            dt = BF16 if arr.dtype == NP_BF16 else F32
            w_in[name] = self.inp(name, list(arr.shape), dt)

        out_t = nc.dram_tensor("out", [1, 1], F32, kind="ExternalOutput")

        with tile.TileContext(nc) as tc:
            self._emit(tc, nodes_in, edgesT_in, send_in, oh2_in, rslot_in,
                       updg_in, scat_in, iota_in, w_in, out_t)
        nc.compile()
        return nc

    # ------------------------------------------------------------------
    def _emit(self, tc, nodes_in, edgesT_in, send_in, oh2_in, rslot_in,
              updg_in, scat_in, iota_in, w_in, out_t):
        cfg = self.cfg
        nc = self.nc
        gpc, B, E_pad, slots = cfg["gpc"], cfg["B"], cfg["E_pad"], cfg["slots"]
        D_EDGE, D_NODE, MSG, OUT = cfg["D_EDGE"], cfg["D_NODE"], self.MSG, self.OUT
        npc = cfg["npc"]
        n_subt = B // 512
        n_layers = self.n_layers

        import contextlib
        ctx = contextlib.ExitStack()
        with ctx:
            const_pool = ctx.enter_context(tc.tile_pool(name="const", bufs=1))
            wpool = ctx.enter_context(tc.tile_pool(name="w", bufs=1))
            sb = ctx.enter_context(tc.tile_pool(name="sb", bufs=2))
            gath = ctx.enter_context(tc.tile_pool(name="gath", bufs=3))
            sbT = ctx.enter_context(tc.tile_pool(name="sbT", bufs=1))
            xsT_pool = ctx.enter_context(tc.tile_pool(name="xsTp", bufs=2))
            xnp_pool = ctx.enter_context(tc.tile_pool(name="xnp", bufs=1))
            act = ctx.enter_context(tc.tile_pool(name="act", bufs=2))
            ps = ctx.enter_context(tc.tile_pool(name="ps", bufs=3, space="PSUM"))
            ps_t = ctx.enter_context(tc.tile_pool(name="ps_t", bufs=2, space="PSUM"))
            ps_sc = ctx.enter_context(tc.tile_pool(name="ps_sc", bufs=1, space="PSUM"))
            ps_ag = ctx.enter_context(tc.tile_pool(name="ps_ag", bufs=2, space="PSUM"))

            # ---- constants ----
            identity = const_pool.tile([P, P], BF16, tag="identity", name="identity")
            make_identity(nc, identity[:])
            iota_sb = const_pool.tile([P, P], F32, tag="iota", name="iota")
            nc.sync.dma_start(iota_sb[:], iota_in[:])
            ones_col = const_pool.tile([P, 1], BF16, tag="ones", name="ones")
            nc.vector.memset(ones_col[:], 1.0)

            # ---- kernel-resident index arrays ----
            send_sb = const_pool.tile([P, E_pad // P], I32, tag="send", name="send")
            nc.sync.dma_start(send_sb[:], send_in[:])
            rslot_sb = const_pool.tile([P, E_pad // P], F32, tag="rslot", name="rslot")
            nc.sync.dma_start(rslot_sb[:], rslot_in[:])
            updg_sb = const_pool.tile([P, gpc], I32, tag="updg", name="updg")
            nc.sync.dma_start(updg_sb[:], updg_in[:])
            scat_sb = const_pool.tile([P, gpc], I32, tag="scat", name="scat")
            nc.sync.dma_start(scat_sb[:], scat_in[:])

            # ---- DRAM scratch (raw tensors: indirect DMA needs offset-0 APs) ----
            ag_in = [nc.dram_tensor(f"ag_in{l}", [npc, MSG], BF16)
                     for l in range(n_layers - 1)]
            ag_out = [nc.dram_tensor(f"ag_out{l}", [cfg["N"], MSG], BF16,
                                     addr_space="Shared")
                      for l in range(n_layers - 1)]

            def load_w(name, tag):
                arr = self.pp[name]
                K, M = arr.shape
                tiles = []
                for k0 in range(0, K, P):
                    kk = min(P, K - k0)
                    t = wpool.tile([P, M], BF16, tag=f"{tag}_{k0 // P}",
                                   name=f"{tag}_{k0 // P}")
                    nc.sync.dma_start(t[:kk, :], w_in[name][k0:k0 + kk, :])
                    tiles.append((t, kk))
                return tiles

            def load_b(name, tag):
                arr = self.pp[name]
                t = wpool.tile([P, arr.shape[1]], F32, tag=tag, name=tag)
                nc.sync.dma_start(t[:], w_in[name][:])
                return t

            def mm_stage(rhs_tiles, w_tiles, b_tile, n_width, func, out_tag,
                         out_sb=None, out_col0=0):
                """Feature-major MLP stage: out[m][128, n_width] = func(W.T@rhs + b).

                func=None -> bias-add on DVE (no activation). out_sb writes
                into caller-provided tiles at column out_col0."""
                assert len(rhs_tiles) == len(w_tiles), (len(rhs_tiles), len(w_tiles))
                M_tot = w_tiles[0][0].shape[1]
                mch = ceil_div(M_tot, P)
                outs = []
                for m in range(mch):
                    mm0 = m * P
                    mw = min(P, M_tot - mm0)
                    pt = ps.tile([P, 512], F32, tag="ps", name="ps")
                    for ki, ent in enumerate(rhs_tiles):
                        rt, kk, col0 = ent[0], ent[1], ent[2] + (
                            ent[3] if len(ent) > 3 else 0)
                        wt, wkk = w_tiles[ki]
                        assert wkk == kk, (wkk, kk)
                        nc.tensor.matmul(
                            pt[:mw, :n_width],
                            lhsT=wt[:kk, mm0:mm0 + mw],
                            rhs=rt[:kk, col0:col0 + n_width],
                            start=(ki == 0), stop=(ki == len(rhs_tiles) - 1))
                    if out_sb is None:
                        o = act.tile([P, 512], BF16, tag=f"{out_tag}{m}",
                                     name=f"{out_tag}{m}")
                        dst = o[:mw, :n_width]
                    else:
                        o = out_sb[m]
                        dst = o[:mw, out_col0:out_col0 + n_width]
                    if func is None:
                        nc.vector.tensor_scalar(
                            dst, pt[:mw, :n_width], b_tile[:mw, m:m + 1], None,
                            ALU.add)
                    else:
                        nc.scalar.activation(dst, pt[:mw, :n_width], func,
                                             bias=b_tile[:mw, m:m + 1])
                    outs.append(o)
                return outs

            def transpose_in(src_tiles, n_rows_tot, d_feat, out_pool, out_tag,
                             out_sb=None, out_col0=0, out_tot=None):
                """Transpose row-major 128-row tiles into feature-major tiles
                [128, out_tot] (one per feat chunk of d_feat)."""
                fch = ceil_div(d_feat, P)
                if out_tot is None:
                    out_tot = n_rows_tot
                outs = []
                for f in range(fch):
                    fw = min(P, d_feat - f * P)
                    if out_sb is None:
                        o = out_pool.tile([P, out_tot], BF16, tag=f"{out_tag}{f}",
                                          name=f"{out_tag}{f}")
                    else:
                        o = out_sb[f]
                    for c0 in range(0, n_rows_tot, 512):
                        cw = min(512, n_rows_tot - c0)
                        pt = ps_t.tile([P, 512], BF16, tag="psT", name="psT")
                        for b0 in range(0, cw, P):
                            bw = min(P, cw - b0)
                            rt = src_tiles[(c0 + b0) // P]
                            nc.tensor.transpose(
                                pt[:fw, b0:b0 + bw],
                                in_=rt[:bw, f * P:f * P + fw],
                                identity=identity[:])
                        nc.vector.tensor_copy(
                            o[:fw, out_col0 + c0:out_col0 + c0 + cw],
                            pt[:fw, :cw])
                    outs.append(o)
                return outs

            def gather_rows(src_dram, idx_col_fn, n_rows, d, tag):
                """Indirect gather of n_rows (multiple of 128) rows of width d."""
                tiles = []
                for r0 in range(0, n_rows, P):
                    c = r0 // P
                    t = gath.tile([P, 256], BF16, tag=f"{tag}{c % 4}",
                                  name=f"{tag}{c % 4}")
                    nc.gpsimd.indirect_dma_start(
                        out=t[:, :d], out_offset=None, in_=src_dram,
                        in_offset=bass.IndirectOffsetOnAxis(ap=idx_col_fn(c), axis=0))
                    tiles.append(t)
                return tiles

            # ================= layers =================
            x_src = nodes_in[:]
            xnewT_last = None
            xnode_of_group = {}

            for l in range(n_layers):
                d_in = D_NODE if l == 0 else MSG
                in_fch = d_in // P

                msg_w = [load_w(f"l{l}_msg{i}_w", f"wm{i}") for i in range(3)]
                msg_b = [load_b(f"l{l}_msg{i}_b", f"bm{i}") for i in range(3)]
                gate_w = [load_w(f"l{l}_gate{i}_w", f"wg{i}") for i in range(2)]
                gate_b = [load_b(f"l{l}_gate{i}_b", f"bg{i}") for i in range(2)]
                attn_w = load_w(f"l{l}_attn_w", "wa")
                attn_b = wpool.tile([P, 1], F32, tag="ab", name="ab")
                nc.vector.memset(attn_b[:], self.pp[f"l{l}_attn_b"])
                upd_w = [load_w(f"l{l}_upd{i}_w", f"wu{i}") for i in range(3)]
                upd_b = [load_b(f"l{l}_upd{i}_b", f"bu{i}") for i in range(3)]

                aggrT = [sbT.tile([P, slots], BF16, tag=f"aggrT{f}",
                                  name=f"aggrT{f}") for f in range(MSG // P)]

                for g in range(gpc):
                    pa = ps_ag.tile([P, MSG + 2], F32, tag="aggr", name="aggr")
                    # group receiver-node features (slot order = updg col g)
                    xnode = xnp_pool.tile([P, 256], BF16, tag=f"xnode{g}",
                                          name=f"xnode{g}")
                    nc.gpsimd.indirect_dma_start(
                        out=xnode[:, :d_in], out_offset=None, in_=x_src,
                        in_offset=bass.IndirectOffsetOnAxis(
                            ap=updg_sb[:, g:g + 1], axis=0))
                    xnode_of_group[g] = xnode
                    msgT_g = [act.tile([P, n_subt * 512], BF16, tag=f"msgg{f}",
                                       name=f"msgg{f}") for f in range(MSG // P)]
                    pt_s = ps_sc.tile([P, 4 * n_subt], F32, tag="sc", name="sc")
                    for s in range(n_subt):
                        t_idx = g * n_subt + s
                        col0 = t_idx * 4
                        xs_g = gather_rows(
                            x_src, lambda c: send_sb[:, col0 + c:col0 + c + 1],
                            512, d_in, "xs")
                        xsT = transpose_in(xs_g, 512, d_in, xsT_pool, "xsT")
                        oh2 = sb.tile([P, 512], BF16, tag="oh2", name="oh2")
                        nc.sync.dma_start(
                            oh2[:], oh2_in[:, t_idx * 512:(t_idx + 1) * 512])
                        # expand receiver features: xrT = xnode.T @ oh2
                        xrT = []
                        for f in range(in_fch):
                            pe_x = ps_t.tile([P, 512], F32, tag="psT", name="psT")
                            nc.tensor.matmul(
                                pe_x[:, :512], lhsT=xnode[:, f * P:(f + 1) * P],
                                rhs=oh2[:], start=True, stop=True)
                            xo = xsT_pool.tile([P, 512], BF16, tag=f"xrT{f}",
                                               name=f"xrT{f}")
                            nc.vector.tensor_copy(xo[:], pe_x[:, :512])
                            xrT.append(xo)
                        edg = sb.tile([D_EDGE, 512], BF16, tag="edg", name="edg")
                        nc.sync.dma_start(
                            edg[:], edgesT_in[:, t_idx * 512:(t_idx + 1) * 512])

                        rhs1 = ([(t, P, 0) for t in xsT] + [(t, P, 0) for t in xrT]
                                + [(edg, D_EDGE, 0)])
                        h1 = mm_stage(rhs1, msg_w[0], msg_b[0], 512, AF.Silu, "h1")
                        h2 = mm_stage([(t, P, 0) for t in h1], msg_w[1], msg_b[1],
                                      512, AF.Silu, "h2")
                        msgsT = mm_stage([(t, P, 0) for t in h2], msg_w[2], msg_b[2],
                                         512, None, "ms", out_sb=msgT_g,
                                         out_col0=s * 512)
                        g1 = mm_stage([(t, P, 0, s * 512) for t in msgT_g],
                                      gate_w[0], gate_b[0], 512, AF.Silu, "g1")
                        g2 = mm_stage([(t, P, 0) for t in g1], gate_w[1],
                                      gate_b[1], 512, None, "g2")

                        for ec in range(4):
                            nc.tensor.matmul(
                                pt_s[:, s * 4 + ec:s * 4 + ec + 1],
                                lhsT=g2[0][:, ec * P:(ec + 1) * P],
                                rhs=attn_w[0][0][:, 0:1],
                                start=True, stop=True)
                    e_t = act.tile([P, 4 * n_subt], F32, tag="e", name="e")
                    nc.scalar.activation(e_t[:], pt_s[:], AF.Exp,
                                         bias=attn_b[:, 0:1])

                    for s in range(n_subt):
                        col0 = (g * n_subt + s) * 4
                        for ec in range(4):
                            pm = ps_t.tile([P, 512], BF16, tag="psT", name="psT")
                            for f in range(MSG // P):
                                nc.tensor.transpose(
                                    pm[:, f * P:(f + 1) * P],
                                    in_=msgT_g[f][:, s * 512 + ec * P:
                                                  s * 512 + (ec + 1) * P],
                                    identity=identity[:])
                            me = sb.tile([P, MSG + 2], BF16, tag=f"me{ec % 2}",
                                         name=f"me{ec % 2}")
                            nc.vector.tensor_copy(me[:, :MSG], pm[:, :MSG])
                            nc.vector.tensor_copy(me[:, MSG:MSG + 1], ones_col[:])
                            nc.vector.memset(me[:, MSG + 1:MSG + 2], 0.0)
                            oh = sb.tile([P, P], BF16, tag=f"oh{ec % 2}",
                                         name=f"oh{ec % 2}")
                            nc.vector.tensor_scalar(
                                oh[:], iota_sb[:],
                                rslot_sb[:, col0 + ec:col0 + ec + 1],
                                e_t[:, s * 4 + ec:s * 4 + ec + 1],
                                ALU.is_equal, ALU.mult)
                            nc.tensor.matmul(
                                pa[:], lhsT=oh[:], rhs=me[:],
                                start=(s == 0 and ec == 0),
                                stop=(s == n_subt - 1 and ec == 3))

                    dn = act.tile([P, 1], F32, tag="dn", name="dn")
                    nc.vector.tensor_scalar_add(dn[:], pa[:, MSG:MSG + 1], 1e-30)
                    rc = act.tile([P, 1], F32, tag="rc", name="rc")
                    nc.vector.reciprocal(rc[:], dn[:])
                    agg_nm = act.tile([P, MSG], BF16, tag="aggnm", name="aggnm")
                    nc.scalar.activation(agg_nm[:], pa[:, :MSG], AF.Copy,
                                         scale=rc[:])
                    transpose_in([agg_nm], P, MSG, None, None, out_sb=aggrT,
                                 out_col0=g * P, out_tot=slots)

                # ---- node update ----
                xT = transpose_in([xnode_of_group[g] for g in range(gpc)],
                                  slots, d_in, sbT, "xT")

                xnewT = [sbT.tile([P, slots], BF16, tag=f"xnT{f}", name=f"xnT{f}")
                         for f in range(OUT // P)]
                for c0 in range(0, slots, 512):
                    cw = min(512, slots - c0)
                    rhs_u = ([(t, P, c0) for t in xT] + [(t, P, c0) for t in aggrT])
                    u1 = mm_stage(rhs_u, upd_w[0], upd_b[0], cw, AF.Silu, "h1")
                    u2 = mm_stage([(t, P, 0) for t in u1], upd_w[1], upd_b[1],
                                  cw, AF.Silu, "h2")
                    u3 = mm_stage([(t, P, 0) for t in u2], upd_w[2], upd_b[2],
                                  cw, None, "u3")
                    for f in range(OUT // P):
                        nc.vector.tensor_copy(xnewT[f][:, c0:c0 + cw],
                                              u3[f][:, :cw])
                if l > 0:
                    for f in range(OUT // P):
                        nc.scalar.activation(xT[f][:], xT[f][:], AF.Copy,
                                             scale=0.5)
                        nc.vector.tensor_tensor(
                            xnewT[f][:], xnewT[f][:], xT[f][:], op=ALU.add)

                if l < n_layers - 1:
                    for g in range(gpc):
                        pn = ps_t.tile([P, 512], BF16, tag="psT", name="psT")
                        for f in range(MSG // P):
                            nc.tensor.transpose(
                                pn[:, f * P:(f + 1) * P],
                                in_=xnewT[f][:, g * P:(g + 1) * P],
                                identity=identity[:])
                        xn = sb.tile([P, MSG], BF16, tag=f"xn{g % 2}",
                                     name=f"xn{g % 2}")
                        nc.vector.tensor_copy(xn[:], pn[:, :MSG])
                        nc.gpsimd.indirect_dma_start(
                            out=ag_in[l][:], out_offset=bass.IndirectOffsetOnAxis(
                                ap=scat_sb[:, g:g + 1], axis=0),
                            in_=xn[:], in_offset=None,
                            bounds_check=npc - 1, oob_is_err=False)
                    nc.gpsimd.collective_compute(
                        "AllGather", ALU.bypass,
                        ins=[ag_in[l][:]], outs=[ag_out[l][:]],
                        replica_groups=[list(range(cfg["n_cores"]))])
                    x_src = ag_out[l][:]
                else:
                    xnewT_last = xnewT

            # ================= head (core 0's result is the output) ==========
            s0 = cfg["s0"]
            head_w = [load_w("head0_w", "wm0"), load_w("head1_w", "wm1"),
                      load_w("head2_w", "wm2"), load_w("head3_w", "wa")]
            head_b = [load_b(f"head{i}_b", f"bm{i % 3}") for i in range(4)]
            cur = [(t, P, s0) for t in xnewT_last]
            for hi in range(3):
                M_tot = head_w[hi][0][0].shape[1]
                mch = ceil_div(M_tot, P)
                outs = []
                for m in range(mch):
                    mw = min(P, M_tot - m * P)
                    pt = ps_sc.tile([P, 4], F32, tag="sc", name="sc")
                    for ki, (rt, kk, col0) in enumerate(cur):
                        nc.tensor.matmul(
                            pt[:mw, 0:1],
                            lhsT=head_w[hi][ki][0][:kk, m * P:m * P + mw],
                            rhs=rt[:kk, col0:col0 + 1],
                            start=(ki == 0), stop=(ki == len(cur) - 1))
                    o = act.tile([P, 1], BF16, tag=f"hh{hi}_{m}", name=f"hh{hi}_{m}")
                    nc.scalar.activation(o[:mw, :], pt[:mw, 0:1], AF.Silu,
                                         bias=head_b[hi][:mw, m:m + 1])
                    outs.append((o, mw, 0))
                cur = outs
            pt = ps_sc.tile([P, 4], F32, tag="sc", name="sc")
            assert len(cur) == 1
            nc.tensor.matmul(pt[:1, 0:1],
                             lhsT=head_w[3][0][0][:, 0:1],
                             rhs=cur[0][0][:, 0:1],
                             start=True, stop=True)
            ot = act.tile([P, 1], F32, tag="outt", name="outt")
            nc.scalar.activation(ot[:1, :], pt[:1, 0:1], AF.Tanh,
                                 bias=head_b[3][:1, 0:1])
            nc.sync.dma_start(out_t[:], ot[:1, :])


# ----------------------------------------------------------------------------

def build_and_inputs(nodes, edges, senders, receivers, params, n_cores=8,
                     n_layers=4):
    nodes = np.ascontiguousarray(np.asarray(nodes, np.float32))
    edges = np.ascontiguousarray(np.asarray(edges, np.float32))
    cfg, per_core = preprocess(nodes, edges, senders, receivers, n_cores)
    pp = pack_params(params, n_layers)
    b = Builder(cfg, pp, n_layers)
    nc = b.build()

    iota_mat = np.tile(np.arange(P, dtype=np.float32), (P, 1))
    nodes_bf = nodes.astype(NP_BF16)
    in_maps = []
    for c in range(n_cores):
        m = {"nodes": nodes_bf, "iota_mat": iota_mat}
        pc = per_core[c]
        for k in ("edgesT", "send_idx", "oh2", "recv_slot",
                  "upd_gather", "scat_idx"):
            m[k] = np.ascontiguousarray(pc[k])
        for name, arr in pp.items():
            if isinstance(arr, float):
                continue
            m[name] = np.ascontiguousarray(arr)
        in_maps.append(m)
    return nc, in_maps, cfg


def golden(nodes, edges, senders, receivers, params, n_layers=4):
    """Numpy mirror of the reference model (any sizes)."""
    def apply(d, x):
        return x @ np.asarray(d["w"], np.float32) + np.asarray(d["b"], np.float32)

    def swish(x):
        return x / (1.0 + np.exp(-x))

    N = nodes.shape[0]
    x = np.asarray(nodes, np.float32)
    senders = np.asarray(senders)
    receivers = np.asarray(receivers)
    for l in range(n_layers):
        p = params[f"layer{l}"]
        h = np.concatenate([edges, x[senders], x[receivers]], axis=-1)
        for d in p["msg"][:-1]:
            h = swish(apply(d, h))
        msgs = apply(p["msg"][-1], h)
        g = swish(apply(p["gate"][0], msgs))
        g = apply(p["gate"][1], g)
        scores = apply(p["attn"], g)[:, 0]
        e = np.exp(scores)
        denom = np.zeros(N, np.float32)
        np.add.at(denom, receivers, e)
        attn = e / denom[receivers]
        aggr = np.zeros((N, msgs.shape[1]), np.float32)
        np.add.at(aggr, receivers, attn[:, None] * msgs)
        hh = np.concatenate([x, aggr], axis=-1)
        for d in p["upd"][:-1]:
            hh = swish(apply(d, hh))
        new = apply(p["upd"][-1], hh)
        if new.shape[-1] == x.shape[-1]:
            new = new + x * 0.5
        x = new
    ego = x[0:1]
    for d in params["head"][:-1]:
        ego = swish(apply(d, ego))
    return np.tanh(apply(params["head"][-1], ego))[0, 0]


# ----------------------------------------------------------------------------
# Harness entry point
# ----------------------------------------------------------------------------

N_CORES = 8
N_LAYERS = 4


def kernel(nodes, edges, senders, receivers, params):
    """Full-input entry: shards across 8 NeuronCores, runs the Bass kernel,
    returns the scalar CBF output (matches reference(**inputs))."""
    from concourse.bass_utils import run_bass_kernel_spmd

    nodes = np.asarray(nodes, np.float32)
    edges = np.asarray(edges, np.float32)
    nc, in_maps, cfg = build_and_inputs(nodes, edges, senders, receivers,
                                        params, n_cores=N_CORES,
                                        n_layers=N_LAYERS)
    res = run_bass_kernel_spmd(nc, in_maps, core_ids=list(range(N_CORES)))
    out = np.asarray(res.results[0]["out"], np.float32).reshape(())
    return out


# revision 11
# speedup vs baseline: 1.1693x; 1.1693x over previous
"""GNN message-passing kernel for trn2 (8 NeuronCores, SPMD).

Sharding: nodes split evenly across cores; each edge owned by its receiver's
core. Within a core, nodes are bin-packed into groups of 128 slots with
balanced edge counts; each group's edge list is padded to a fixed budget B
(multiple of 512). Segment softmax + aggregation are core-local (one-hot
matmul per group; shiftless exp is numerically safe for this model's score
range). Node-update MLP is data-parallel; results are scattered into a
per-core staging buffer and AllGathered into the replicated x.

Layouts: activations flow feature-major [feat(part), item(free)] through all
MLP matmuls (lhsT = weight [K=din, M=dout], rhs = activation). Gathered rows
arrive item-major and are PE-transposed. Matmul operands are bf16 (FWL fast
weight loads, full-rate PE); PSUM accumulation, biases and the softmax are
fp32.
"""
import sys, types
import numpy as np
import ml_dtypes

import concourse.bass as bass
import concourse.tile as tile
from concourse import bacc, mybir
from concourse.masks import make_identity

F32 = mybir.dt.float32
BF16 = mybir.dt.bfloat16
I32 = mybir.dt.int32
AF = mybir.ActivationFunctionType
ALU = mybir.AluOpType

P = 128
NP_BF16 = ml_dtypes.bfloat16


def install_ntff_shim():
    """The agent image lacks antenv.axon_hooks; install a shim so trace=True works."""
    if "antenv.axon_hooks" in sys.modules:
        return
    import antenv
    _hooks = types.ModuleType("antenv.axon_hooks")
    _hooks._hook = None
    def _set(h):
        _hooks._hook = h
    def _get():
        return _hooks._hook
    _hooks.set_axon_ntff_profile_hook = _set
    _hooks.get_axon_ntff_profile_hook = _get
    sys.modules["antenv.axon_hooks"] = _hooks
    antenv.axon_hooks = _hooks
    try:
        from trn_agent_boot.trn_boot import _ntff_profile_via_ctypes
        _set(_ntff_profile_via_ctypes("/opt/axon/libaxon_pjrt.so"))
    except Exception:
        pass


def ceil_div(a, b):
    return (a + b - 1) // b


# ----------------------------------------------------------------------------
# Host preprocessing
# ----------------------------------------------------------------------------

def preprocess(nodes, edges, senders, receivers, n_cores):
    """Shard + permute. Returns (cfg, per_core_inputs:list[dict])."""
    N, D_NODE = nodes.shape
    E, D_EDGE = edges.shape
    senders = np.asarray(senders).astype(np.int64)
    receivers = np.asarray(receivers).astype(np.int64)
    assert N % n_cores == 0
    npc = N // n_cores                      # nodes per core
    gpc = ceil_div(npc, P)                  # groups per core
    slots = gpc * P

    deg = np.bincount(receivers, minlength=N)
    edges_of_core = [np.where(receivers // npc == c)[0] for c in range(n_cores)]

    # Bin-pack each core's nodes into gpc groups of <=128 slots, balancing edges.
    group_of_node = np.full(N, -1, np.int64)
    slot_of_node = np.full(N, -1, np.int64)
    max_group_edges = 0
    for c in range(n_cores):
        local = np.arange(npc * c, npc * (c + 1))
        order = local[np.argsort(-deg[local], kind="stable")]
        gsum = np.zeros(gpc, np.int64)
        gcnt = np.zeros(gpc, np.int64)
        for n in order:
            cand = np.where(gcnt < P)[0]
            g = cand[np.argmin(gsum[cand])]
            group_of_node[n] = g
            slot_of_node[n] = gcnt[g]
            gsum[g] += deg[n]
            gcnt[g] += 1
        max_group_edges = max(max_group_edges, int(gsum.max()))

    B = max(512, ceil_div(max_group_edges, 512) * 512)   # edge budget per group
    E_pad = gpc * B

    per_core = []
    for c in range(n_cores):
        eix = edges_of_core[c]
        g_of_e = group_of_node[receivers[eix]]
        send_idx = np.zeros(E_pad, np.int64)
        recv_idx = np.zeros(E_pad, np.int64)
        recv_slot = np.full(E_pad, 255.0, np.float32)
        edge_perm = np.full(E_pad, -1, np.int64)
        for g in range(gpc):
            ge = eix[g_of_e == g]
            assert len(ge) <= B, f"group overflow {len(ge)} > {B}"
            base = g * B
            send_idx[base:base + len(ge)] = senders[ge]
            recv_idx[base:base + len(ge)] = receivers[ge]
            recv_slot[base:base + len(ge)] = slot_of_node[receivers[ge]].astype(np.float32)
            edge_perm[base:base + len(ge)] = ge

        edgesT = np.zeros((D_EDGE, E_pad), np.float32)
        real = edge_perm >= 0
        edgesT[:, real] = np.asarray(edges, np.float32)[edge_perm[real]].T

        upd_gather = np.zeros(slots, np.int64)
        scat_idx = np.full(slots, 2_000_000, np.int64)
        local = np.arange(npc * c, npc * (c + 1))
        s_of = group_of_node[local] * P + slot_of_node[local]
        upd_gather[s_of] = local
        scat_idx[s_of] = local - npc * c

        oh2 = np.zeros((P, E_pad), np.float32)
        real_j = np.where(recv_slot < P)[0]
        oh2[recv_slot[real_j].astype(np.int64), real_j] = 1.0

        per_core.append({
            "send_idx": send_idx.astype(np.int32).reshape(-1, P).T.copy(),
            "oh2": oh2.astype(NP_BF16),
            "recv_slot": recv_slot.reshape(-1, P).T.copy(),
            "edgesT": edgesT.astype(NP_BF16),
            "upd_gather": upd_gather.astype(np.int32).reshape(-1, P).T.copy(),
            "scat_idx": scat_idx.astype(np.int32).reshape(-1, P).T.copy(),
        })

    s0 = int(group_of_node[0] * P + slot_of_node[0])
    cfg = dict(N=N, E=E, D_NODE=D_NODE, D_EDGE=D_EDGE, n_cores=n_cores,
               npc=npc, gpc=gpc, slots=slots, B=B, E_pad=E_pad, s0=s0)
    return cfg, per_core


def pack_params(params, n_layers):
    """Flatten params into {name: np.ndarray|float}. Weights are bf16 in
    matmul lhsT layout [din, dout]; biases are f32 [128, mch].

    msg0 weight rows are reordered from [edge, xs, xr] to [xs, xr, edge] so
    that 128-row K-chunks align with the rhs source tiles.
    """
    out = {}

    def put_w(name, w):
        out[name] = np.ascontiguousarray(np.asarray(w, np.float32)).astype(NP_BF16)

    def put_b(name, b):
        b = np.asarray(b, np.float32)
        dout = b.shape[0]
        mch = ceil_div(dout, P)
        bp = np.zeros((mch * P,), np.float32)
        bp[:dout] = b
        out[name] = bp.reshape(mch, P).T.copy()

    def put_mlp(prefix, mlp):
        for i, d in enumerate(mlp):
            put_w(f"{prefix}{i}_w", d["w"])
            put_b(f"{prefix}{i}_b", d["b"])

    for l in range(n_layers):
        p = params[f"layer{l}"]
        put_mlp(f"l{l}_msg", p["msg"])
        w1 = np.asarray(p["msg"][0]["w"], np.float32)
        d_in = (w1.shape[0] - 16) // 2  # D_EDGE = 16 for this model family
        d_e = w1.shape[0] - 2 * d_in
        out[f"l{l}_msg0_w"] = np.ascontiguousarray(
            np.concatenate([w1[d_e:d_e + d_in], w1[d_e + d_in:], w1[:d_e]],
                           axis=0)).astype(NP_BF16)
        put_mlp(f"l{l}_gate", p["gate"])
        put_w(f"l{l}_attn_w", p["attn"]["w"])
        out[f"l{l}_attn_b"] = float(np.asarray(p["attn"]["b"]).reshape(()))
        put_mlp(f"l{l}_upd", p["upd"])
    put_mlp("head", params["head"])
    return out


# ----------------------------------------------------------------------------
# Bass program builder
# ----------------------------------------------------------------------------

class Builder:
    def __init__(self, cfg, pp, n_layers):
        self.cfg = cfg
        self.pp = pp
        self.n_layers = n_layers
        self.MSG = pp["l0_msg2_w"].shape[1]
        self.OUT = pp[f"l{n_layers-1}_upd2_w"].shape[1]
        self.nc = bacc.Bacc("TRN2", target_bir_lowering=False, debug=False,
                            num_devices=cfg["n_cores"])
        self.input_names = []

    def inp(self, name, shape, dtype):
        h = self.nc.dram_tensor(name, list(shape), dtype, kind="ExternalInput")
        self.input_names.append(name)
        return h

    def build(self):
        cfg = self.cfg
        nc = self.nc
        gpc, E_pad = cfg["gpc"], cfg["E_pad"]
        D_EDGE, D_NODE = cfg["D_EDGE"], cfg["D_NODE"]

        nodes_in = self.inp("nodes", [cfg["N"], D_NODE], BF16)
        edgesT_in = self.inp("edgesT", [D_EDGE, E_pad], BF16)
        send_in = self.inp("send_idx", [P, E_pad // P], I32)
        oh2_in = self.inp("oh2", [P, E_pad], BF16)
        rslot_in = self.inp("recv_slot", [P, E_pad // P], F32)
        updg_in = self.inp("upd_gather", [P, gpc], I32)
        scat_in = self.inp("scat_idx", [P, gpc], I32)
        iota_in = self.inp("iota_mat", [P, P], F32)

        w_in = {}
        for name, arr in self.pp.items():
            if isinstance(arr, float):
                continue
            dt = BF16 if arr.dtype == NP_BF16 else F32
            w_in[name] = self.inp(name, list(arr.shape), dt)

        out_t = nc.dram_tensor("out", [1, 1], F32, kind="ExternalOutput")

        with tile.TileContext(nc) as tc:
            self._emit(tc, nodes_in, edgesT_in, send_in, oh2_in, rslot_in,
                       updg_in, scat_in, iota_in, w_in, out_t)
        nc.compile()
        return nc

    # ------------------------------------------------------------------
    def _emit(self, tc, nodes_in, edgesT_in, send_in, oh2_in, rslot_in,
              updg_in, scat_in, iota_in, w_in, out_t):
        cfg = self.cfg
        nc = self.nc
        gpc, B, E_pad, slots = cfg["gpc"], cfg["B"], cfg["E_pad"], cfg["slots"]
        D_EDGE, D_NODE, MSG, OUT = cfg["D_EDGE"], cfg["D_NODE"], self.MSG, self.OUT
        npc = cfg["npc"]
        n_subt = B // 512
        n_layers = self.n_layers

        import contextlib
        ctx = contextlib.ExitStack()
        with ctx:
            const_pool = ctx.enter_context(tc.tile_pool(name="const", bufs=1))
            wpool = ctx.enter_context(tc.tile_pool(name="w", bufs=1))
            sb = ctx.enter_context(tc.tile_pool(name="sb", bufs=2))
            gath = ctx.enter_context(tc.tile_pool(name="gath", bufs=3))
            sbT = ctx.enter_context(tc.tile_pool(name="sbT", bufs=1))
            xsT_pool = ctx.enter_context(tc.tile_pool(name="xsTp", bufs=2))
            xnp_pool = ctx.enter_context(tc.tile_pool(name="xnp", bufs=1))
            act = ctx.enter_context(tc.tile_pool(name="act", bufs=2))
            ps = ctx.enter_context(tc.tile_pool(name="ps", bufs=3, space="PSUM"))
            ps_t = ctx.enter_context(tc.tile_pool(name="ps_t", bufs=2, space="PSUM"))
            ps_sc = ctx.enter_context(tc.tile_pool(name="ps_sc", bufs=1, space="PSUM"))
            ps_ag = ctx.enter_context(tc.tile_pool(name="ps_ag", bufs=2, space="PSUM"))

            # ---- constants ----
            identity = const_pool.tile([P, P], BF16, tag="identity", name="identity")
            make_identity(nc, identity[:])
            iota_sb = const_pool.tile([P, P], F32, tag="iota", name="iota")
            nc.sync.dma_start(iota_sb[:], iota_in[:])
            ones_col = const_pool.tile([P, 1], BF16, tag="ones", name="ones")
            nc.vector.memset(ones_col[:], 1.0)

            # ---- kernel-resident index arrays ----
            send_sb = const_pool.tile([P, E_pad // P], I32, tag="send", name="send")
            nc.sync.dma_start(send_sb[:], send_in[:])
            rslot_sb = const_pool.tile([P, E_pad // P], F32, tag="rslot", name="rslot")
            nc.sync.dma_start(rslot_sb[:], rslot_in[:])
            updg_sb = const_pool.tile([P, gpc], I32, tag="updg", name="updg")
            nc.sync.dma_start(updg_sb[:], updg_in[:])
            scat_sb = const_pool.tile([P, gpc], I32, tag="scat", name="scat")
            nc.sync.dma_start(scat_sb[:], scat_in[:])

            # ---- DRAM scratch (raw tensors: indirect DMA needs offset-0 APs) ----
            ag_in = [nc.dram_tensor(f"ag_in{l}", [npc, MSG], BF16)
                     for l in range(n_layers - 1)]
            ag_out = [nc.dram_tensor(f"ag_out{l}", [cfg["N"], MSG], BF16,
                                     addr_space="Shared")
                      for l in range(n_layers - 1)]

            def load_w(name, tag):
                arr = self.pp[name]
                K, M = arr.shape
                tiles = []
                for k0 in range(0, K, P):
                    kk = min(P, K - k0)
                    t = wpool.tile([P, M], BF16, tag=f"{tag}_{k0 // P}",
                                   name=f"{tag}_{k0 // P}")
                    nc.sync.dma_start(t[:kk, :], w_in[name][k0:k0 + kk, :])
                    tiles.append((t, kk))
                return tiles

            def load_b(name, tag):
                arr = self.pp[name]
                t = wpool.tile([P, arr.shape[1]], F32, tag=tag, name=tag)
                nc.sync.dma_start(t[:], w_in[name][:])
                return t

            def mm_stage(rhs_tiles, w_tiles, b_tile, n_width, func, out_tag,
                         out_sb=None, out_col0=0):
                """Feature-major MLP stage: out[m][128, n_width] = func(W.T@rhs + b).

                func=None -> bias-add on DVE (no activation). out_sb writes
                into caller-provided tiles at column out_col0."""
                assert len(rhs_tiles) == len(w_tiles), (len(rhs_tiles), len(w_tiles))
                M_tot = w_tiles[0][0].shape[1]
                mch = ceil_div(M_tot, P)
                outs = []
                for m in range(mch):
                    mm0 = m * P
                    mw = min(P, M_tot - mm0)
                    pt = ps.tile([P, 512], F32, tag="ps", name="ps")
                    for ki, ent in enumerate(rhs_tiles):
                        rt, kk, col0 = ent[0], ent[1], ent[2] + (
                            ent[3] if len(ent) > 3 else 0)
                        wt, wkk = w_tiles[ki]
                        assert wkk == kk, (wkk, kk)
                        nc.tensor.matmul(
                            pt[:mw, :n_width],
                            lhsT=wt[:kk, mm0:mm0 + mw],
                            rhs=rt[:kk, col0:col0 + n_width],
                            start=(ki == 0), stop=(ki == len(rhs_tiles) - 1))
                    if out_sb is None:
                        o = act.tile([P, 512], BF16, tag=f"{out_tag}{m}",
                                     name=f"{out_tag}{m}")
                        dst = o[:mw, :n_width]
                    else:
                        o = out_sb[m]
                        dst = o[:mw, out_col0:out_col0 + n_width]
                    if func is None:
                        nc.vector.tensor_scalar(
                            dst, pt[:mw, :n_width], b_tile[:mw, m:m + 1], None,
                            ALU.add)
                    else:
                        nc.scalar.activation(dst, pt[:mw, :n_width], func,
                                             bias=b_tile[:mw, m:m + 1])
                    outs.append(o)
                return outs

            def transpose_in(src_tiles, n_rows_tot, d_feat, out_pool, out_tag,
                             out_sb=None, out_col0=0, out_tot=None):
                """Transpose row-major 128-row tiles into feature-major tiles
                [128, out_tot] (one per feat chunk of d_feat)."""
                fch = ceil_div(d_feat, P)
                if out_tot is None:
                    out_tot = n_rows_tot
                outs = []
                for f in range(fch):
                    fw = min(P, d_feat - f * P)
                    if out_sb is None:
                        o = out_pool.tile([P, out_tot], BF16, tag=f"{out_tag}{f}",
                                          name=f"{out_tag}{f}")
                    else:
                        o = out_sb[f]
                    for c0 in range(0, n_rows_tot, 512):
                        cw = min(512, n_rows_tot - c0)
                        pt = ps_t.tile([P, 512], BF16, tag="psT", name="psT")
                        for b0 in range(0, cw, P):
                            bw = min(P, cw - b0)
                            rt = src_tiles[(c0 + b0) // P]
                            nc.tensor.transpose(
                                pt[:fw, b0:b0 + bw],
                                in_=rt[:bw, f * P:f * P + fw],
                                identity=identity[:])
                        nc.vector.tensor_copy(
                            o[:fw, out_col0 + c0:out_col0 + c0 + cw],
                            pt[:fw, :cw])
                    outs.append(o)
                return outs

            def gather_rows(src_dram, idx_col_fn, n_rows, d, tag):
                """Indirect gather of n_rows (multiple of 128) rows of width d."""
                tiles = []
                for r0 in range(0, n_rows, P):
                    c = r0 // P
                    t = gath.tile([P, 256], BF16, tag=f"{tag}{c % 4}",
                                  name=f"{tag}{c % 4}")
                    nc.gpsimd.indirect_dma_start(
                        out=t[:, :d], out_offset=None, in_=src_dram,
                        in_offset=bass.IndirectOffsetOnAxis(ap=idx_col_fn(c), axis=0))
                    tiles.append(t)
                return tiles

            # ================= layers =================
            x_src = nodes_in[:]
            xnewT_last = None
            xnode_of_group = {}

            for l in range(n_layers):
                d_in = D_NODE if l == 0 else MSG
                in_fch = d_in // P

                msg_w = [load_w(f"l{l}_msg{i}_w", f"wm{i}") for i in range(3)]
                msg_b = [load_b(f"l{l}_msg{i}_b", f"bm{i}") for i in range(3)]
                gate_w = [load_w(f"l{l}_gate{i}_w", f"wg{i}") for i in range(2)]
                gate_b = [load_b(f"l{l}_gate{i}_b", f"bg{i}") for i in range(2)]
                attn_w = load_w(f"l{l}_attn_w", "wa")
                attn_b = wpool.tile([P, 1], F32, tag="ab", name="ab")
                nc.vector.memset(attn_b[:], self.pp[f"l{l}_attn_b"])
                upd_w = [load_w(f"l{l}_upd{i}_w", f"wu{i}") for i in range(3)]
                upd_b = [load_b(f"l{l}_upd{i}_b", f"bu{i}") for i in range(3)]

                aggrT = [sbT.tile([P, slots], BF16, tag=f"aggrT{f}",
                                  name=f"aggrT{f}") for f in range(MSG // P)]

                def emit_aggregation(p):
                    pa_, chunks_, g_ = p
                    for ci, (oh_, me_) in enumerate(chunks_):
                        nc.tensor.matmul(
                            pa_[:], lhsT=oh_[:], rhs=me_[:],
                            start=(ci == 0), stop=(ci == len(chunks_) - 1))
                    dn = act.tile([P, 1], F32, tag="dn", name="dn")
                    nc.vector.tensor_scalar_add(dn[:], pa_[:, MSG:MSG + 1], 1e-30)
                    rc = act.tile([P, 1], F32, tag="rc", name="rc")
                    nc.vector.reciprocal(rc[:], dn[:])
                    agg_nm = act.tile([P, MSG], BF16, tag="aggnm", name="aggnm")
                    nc.scalar.activation(agg_nm[:], pa_[:, :MSG], AF.Copy,
                                         scale=rc[:])
                    transpose_in([agg_nm], P, MSG, None, None, out_sb=aggrT,
                                 out_col0=g_ * P, out_tot=slots)

                pend = None
                for g in range(gpc):
                    pa = ps_ag.tile([P, MSG + 2], F32, tag="aggr", name="aggr")
                    # group receiver-node features (slot order = updg col g)
                    xnode = xnp_pool.tile([P, 256], BF16, tag=f"xnode{g}",
                                          name=f"xnode{g}")
                    nc.gpsimd.indirect_dma_start(
                        out=xnode[:, :d_in], out_offset=None, in_=x_src,
                        in_offset=bass.IndirectOffsetOnAxis(
                            ap=updg_sb[:, g:g + 1], axis=0))
                    xnode_of_group[g] = xnode
                    msgT_g = [act.tile([P, n_subt * 512], BF16, tag=f"msgg{f}",
                                       name=f"msgg{f}") for f in range(MSG // P)]
                    pt_s = ps_sc.tile([P, 4 * n_subt], F32, tag="sc", name="sc")
                    for s in range(n_subt):
                        t_idx = g * n_subt + s
                        col0 = t_idx * 4
                        xs_g = gather_rows(
                            x_src, lambda c: send_sb[:, col0 + c:col0 + c + 1],
                            512, d_in, "xs")
                        xsT = transpose_in(xs_g, 512, d_in, xsT_pool, "xsT")
                        oh2 = sb.tile([P, 512], BF16, tag="oh2", name="oh2")
                        nc.sync.dma_start(
                            oh2[:], oh2_in[:, t_idx * 512:(t_idx + 1) * 512])
                        # expand receiver features: xrT = xnode.T @ oh2
                        xrT = []
                        for f in range(in_fch):
                            pe_x = ps_t.tile([P, 512], F32, tag="psT", name="psT")
                            nc.tensor.matmul(
                                pe_x[:, :512], lhsT=xnode[:, f * P:(f + 1) * P],
                                rhs=oh2[:], start=True, stop=True)
                            xo = xsT_pool.tile([P, 512], BF16, tag=f"xrT{f}",
                                               name=f"xrT{f}")
                            nc.vector.tensor_copy(xo[:], pe_x[:, :512])
                            xrT.append(xo)
                        edg = sb.tile([D_EDGE, 512], BF16, tag="edg", name="edg")
                        nc.sync.dma_start(
                            edg[:], edgesT_in[:, t_idx * 512:(t_idx + 1) * 512])

                        rhs1 = ([(t, P, 0) for t in xsT] + [(t, P, 0) for t in xrT]
                                + [(edg, D_EDGE, 0)])
                        h1 = mm_stage(rhs1, msg_w[0], msg_b[0], 512, AF.Silu, "h1")
                        h2 = mm_stage([(t, P, 0) for t in h1], msg_w[1], msg_b[1],
                                      512, AF.Silu, "h2")
                        msgsT = mm_stage([(t, P, 0) for t in h2], msg_w[2], msg_b[2],
                                         512, None, "ms", out_sb=msgT_g,
                                         out_col0=s * 512)
                        g1 = mm_stage([(t, P, 0, s * 512) for t in msgT_g],
                                      gate_w[0], gate_b[0], 512, AF.Silu, "g1")
                        g2 = mm_stage([(t, P, 0) for t in g1], gate_w[1],
                                      gate_b[1], 512, None, "g2")

                        for ec in range(4):
                            nc.tensor.matmul(
                                pt_s[:, s * 4 + ec:s * 4 + ec + 1],
                                lhsT=g2[0][:, ec * P:(ec + 1) * P],
                                rhs=attn_w[0][0][:, 0:1],
                                start=True, stop=True)
                    e_t = act.tile([P, 4 * n_subt], F32, tag="e", name="e")
                    nc.scalar.activation(e_t[:], pt_s[:], AF.Exp,
                                         bias=attn_b[:, 0:1])

                    # build scaled one-hots + edge-major msgs now (DVE);
                    # the aggregation matmuls run one group later so the PE
                    # never waits on this DVE work (software pipelining).
                    chunks = []
                    for s in range(n_subt):
                        col0 = (g * n_subt + s) * 4
                        for ec in range(4):
                            pm = ps_t.tile([P, 512], BF16, tag="psT", name="psT")
                            for f in range(MSG // P):
                                nc.tensor.transpose(
                                    pm[:, f * P:(f + 1) * P],
                                    in_=msgT_g[f][:, s * 512 + ec * P:
                                                  s * 512 + (ec + 1) * P],
                                    identity=identity[:])
                            me = sb.tile([P, MSG + 2], BF16, tag=f"me{s}_{ec}",
                                         name=f"me{s}_{ec}")
                            nc.vector.tensor_copy(me[:, :MSG], pm[:, :MSG])
                            nc.vector.tensor_copy(me[:, MSG:MSG + 1], ones_col[:])
                            nc.vector.memset(me[:, MSG + 1:MSG + 2], 0.0)
                            oh = sb.tile([P, P], BF16, tag=f"oh{s}_{ec}",
                                         name=f"oh{s}_{ec}")
                            nc.vector.tensor_scalar(
                                oh[:], iota_sb[:],
                                rslot_sb[:, col0 + ec:col0 + ec + 1],
                                e_t[:, s * 4 + ec:s * 4 + ec + 1],
                                ALU.is_equal, ALU.mult)
                            chunks.append((oh, me))
                    if pend is not None:
                        emit_aggregation(pend)
                    pend = (pa, chunks, g)

                if pend is not None:
                    emit_aggregation(pend)
                    pend = None

                # ---- node update ----
                xT = transpose_in([xnode_of_group[g] for g in range(gpc)],
                                  slots, d_in, sbT, "xT")

                xnewT = [sbT.tile([P, slots], BF16, tag=f"xnT{f}", name=f"xnT{f}")
                         for f in range(OUT // P)]
                for c0 in range(0, slots, 512):
                    cw = min(512, slots - c0)
                    rhs_u = ([(t, P, c0) for t in xT] + [(t, P, c0) for t in aggrT])
                    u1 = mm_stage(rhs_u, upd_w[0], upd_b[0], cw, AF.Silu, "h1")
                    u2 = mm_stage([(t, P, 0) for t in u1], upd_w[1], upd_b[1],
                                  cw, AF.Silu, "h2")
                    u3 = mm_stage([(t, P, 0) for t in u2], upd_w[2], upd_b[2],
                                  cw, None, "u3")
                    for f in range(OUT // P):
                        nc.vector.tensor_copy(xnewT[f][:, c0:c0 + cw],
                                              u3[f][:, :cw])
                if l > 0:
                    for f in range(OUT // P):
                        nc.scalar.activation(xT[f][:], xT[f][:], AF.Copy,
                                             scale=0.5)
                        nc.vector.tensor_tensor(
                            xnewT[f][:], xnewT[f][:], xT[f][:], op=ALU.add)

                if l < n_layers - 1:
                    for g in range(gpc):
                        pn = ps_t.tile([P, 512], BF16, tag="psT", name="psT")
                        for f in range(MSG // P):
                            nc.tensor.transpose(
                                pn[:, f * P:(f + 1) * P],
                                in_=xnewT[f][:, g * P:(g + 1) * P],
                                identity=identity[:])
                        xn = sb.tile([P, MSG], BF16, tag=f"xn{g % 2}",
                                     name=f"xn{g % 2}")
                        nc.vector.tensor_copy(xn[:], pn[:, :MSG])
                        nc.gpsimd.indirect_dma_start(
                            out=ag_in[l][:], out_offset=bass.IndirectOffsetOnAxis(
                                ap=scat_sb[:, g:g + 1], axis=0),
                            in_=xn[:], in_offset=None,
                            bounds_check=npc - 1, oob_is_err=False)
                    nc.gpsimd.collective_compute(
                        "AllGather", ALU.bypass,
                        ins=[ag_in[l][:]], outs=[ag_out[l][:]],
                        replica_groups=[list(range(cfg["n_cores"]))])
                    x_src = ag_out[l][:]
                else:
                    xnewT_last = xnewT

            # ================= head (core 0's result is the output) ==========
            s0 = cfg["s0"]
            head_w = [load_w("head0_w", "wm0"), load_w("head1_w", "wm1"),
                      load_w("head2_w", "wm2"), load_w("head3_w", "wa")]
            head_b = [load_b(f"head{i}_b", f"bm{i % 3}") for i in range(4)]
            cur = [(t, P, s0) for t in xnewT_last]
            for hi in range(3):
                M_tot = head_w[hi][0][0].shape[1]
                mch = ceil_div(M_tot, P)
                outs = []
                for m in range(mch):
                    mw = min(P, M_tot - m * P)
                    pt = ps_sc.tile([P, 4], F32, tag="sc", name="sc")
                    for ki, (rt, kk, col0) in enumerate(cur):
                        nc.tensor.matmul(
                            pt[:mw, 0:1],
                            lhsT=head_w[hi][ki][0][:kk, m * P:m * P + mw],
                            rhs=rt[:kk, col0:col0 + 1],
                            start=(ki == 0), stop=(ki == len(cur) - 1))
                    o = act.tile([P, 1], BF16, tag=f"hh{hi}_{m}", name=f"hh{hi}_{m}")
                    nc.scalar.activation(o[:mw, :], pt[:mw, 0:1], AF.Silu,
                                         bias=head_b[hi][:mw, m:m + 1])
                    outs.append((o, mw, 0))
                cur = outs
            pt = ps_sc.tile([P, 4], F32, tag="sc", name="sc")
            assert len(cur) == 1
            nc.tensor.matmul(pt[:1, 0:1],
                             lhsT=head_w[3][0][0][:, 0:1],
                             rhs=cur[0][0][:, 0:1],
                             start=True, stop=True)
            ot = act.tile([P, 1], F32, tag="outt", name="outt")
            nc.scalar.activation(ot[:1, :], pt[:1, 0:1], AF.Tanh,
                                 bias=head_b[3][:1, 0:1])
            nc.sync.dma_start(out_t[:], ot[:1, :])


# ----------------------------------------------------------------------------

def build_and_inputs(nodes, edges, senders, receivers, params, n_cores=8,
                     n_layers=4):
    nodes = np.ascontiguousarray(np.asarray(nodes, np.float32))
    edges = np.ascontiguousarray(np.asarray(edges, np.float32))
    cfg, per_core = preprocess(nodes, edges, senders, receivers, n_cores)
    pp = pack_params(params, n_layers)
    b = Builder(cfg, pp, n_layers)
    nc = b.build()

    iota_mat = np.tile(np.arange(P, dtype=np.float32), (P, 1))
    nodes_bf = nodes.astype(NP_BF16)
    in_maps = []
    for c in range(n_cores):
        m = {"nodes": nodes_bf, "iota_mat": iota_mat}
        pc = per_core[c]
        for k in ("edgesT", "send_idx", "oh2", "recv_slot",
                  "upd_gather", "scat_idx"):
            m[k] = np.ascontiguousarray(pc[k])
        for name, arr in pp.items():
            if isinstance(arr, float):
                continue
            m[name] = np.ascontiguousarray(arr)
        in_maps.append(m)
    return nc, in_maps, cfg


def golden(nodes, edges, senders, receivers, params, n_layers=4):
    """Numpy mirror of the reference model (any sizes)."""
    def apply(d, x):
        return x @ np.asarray(d["w"], np.float32) + np.asarray(d["b"], np.float32)

    def swish(x):
        return x / (1.0 + np.exp(-x))

    N = nodes.shape[0]
    x = np.asarray(nodes, np.float32)
    senders = np.asarray(senders)
    receivers = np.asarray(receivers)
    for l in range(n_layers):
        p = params[f"layer{l}"]
        h = np.concatenate([edges, x[senders], x[receivers]], axis=-1)
        for d in p["msg"][:-1]:
            h = swish(apply(d, h))
        msgs = apply(p["msg"][-1], h)
        g = swish(apply(p["gate"][0], msgs))
        g = apply(p["gate"][1], g)
        scores = apply(p["attn"], g)[:, 0]
        e = np.exp(scores)
        denom = np.zeros(N, np.float32)
        np.add.at(denom, receivers, e)
        attn = e / denom[receivers]
        aggr = np.zeros((N, msgs.shape[1]), np.float32)
        np.add.at(aggr, receivers, attn[:, None] * msgs)
        hh = np.concatenate([x, aggr], axis=-1)
        for d in p["upd"][:-1]:
            hh = swish(apply(d, hh))
        new = apply(p["upd"][-1], hh)
        if new.shape[-1] == x.shape[-1]:
            new = new + x * 0.5
        x = new
    ego = x[0:1]
    for d in params["head"][:-1]:
        ego = swish(apply(d, ego))
    return np.tanh(apply(params["head"][-1], ego))[0, 0]


# ----------------------------------------------------------------------------
# Harness entry point
# ----------------------------------------------------------------------------

N_CORES = 8
N_LAYERS = 4


def kernel(nodes, edges, senders, receivers, params):
    """Full-input entry: shards across 8 NeuronCores, runs the Bass kernel,
    returns the scalar CBF output (matches reference(**inputs))."""
    from concourse.bass_utils import run_bass_kernel_spmd

    nodes = np.asarray(nodes, np.float32)
    edges = np.asarray(edges, np.float32)
    nc, in_maps, cfg = build_and_inputs(nodes, edges, senders, receivers,
                                        params, n_cores=N_CORES,
                                        n_layers=N_LAYERS)
    res = run_bass_kernel_spmd(nc, in_maps, core_ids=list(range(N_CORES)))
    out = np.asarray(res.results[0]["out"], np.float32).reshape(())
    return out


# revision 12
# speedup vs baseline: 1.1851x; 1.0135x over previous
"""GNN message-passing kernel for trn2 (8 NeuronCores, SPMD).

Sharding: nodes split evenly across cores; each edge owned by its receiver's
core. Within a core, nodes are bin-packed into groups of 128 slots with
balanced edge counts; each group's edge list is padded to a fixed budget B
(multiple of 512). Segment softmax + aggregation are core-local (one-hot
matmul per group; shiftless exp is numerically safe for this model's score
range). Node-update MLP is data-parallel; results are scattered into a
per-core staging buffer and AllGathered into the replicated x.

Layouts: activations flow feature-major [feat(part), item(free)] through all
MLP matmuls (lhsT = weight [K=din, M=dout], rhs = activation). Gathered rows
arrive item-major and are PE-transposed. Matmul operands are bf16 (FWL fast
weight loads, full-rate PE); PSUM accumulation, biases and the softmax are
fp32.
"""
import sys, types
import numpy as np
import ml_dtypes

import concourse.bass as bass
import concourse.tile as tile
from concourse import bacc, mybir
from concourse.masks import make_identity

F32 = mybir.dt.float32
BF16 = mybir.dt.bfloat16
I32 = mybir.dt.int32
AF = mybir.ActivationFunctionType
ALU = mybir.AluOpType

P = 128
NP_BF16 = ml_dtypes.bfloat16


def install_ntff_shim():
    """The agent image lacks antenv.axon_hooks; install a shim so trace=True works."""
    if "antenv.axon_hooks" in sys.modules:
        return
    import antenv
    _hooks = types.ModuleType("antenv.axon_hooks")
    _hooks._hook = None
    def _set(h):
        _hooks._hook = h
    def _get():
        return _hooks._hook
    _hooks.set_axon_ntff_profile_hook = _set
    _hooks.get_axon_ntff_profile_hook = _get
    sys.modules["antenv.axon_hooks"] = _hooks
    antenv.axon_hooks = _hooks
    try:
        from trn_agent_boot.trn_boot import _ntff_profile_via_ctypes
        _set(_ntff_profile_via_ctypes("/opt/axon/libaxon_pjrt.so"))
    except Exception:
        pass


def ceil_div(a, b):
    return (a + b - 1) // b


# ----------------------------------------------------------------------------
# Host preprocessing
# ----------------------------------------------------------------------------

def preprocess(nodes, edges, senders, receivers, n_cores):
    """Shard + permute. Returns (cfg, per_core_inputs:list[dict])."""
    N, D_NODE = nodes.shape
    E, D_EDGE = edges.shape
    senders = np.asarray(senders).astype(np.int64)
    receivers = np.asarray(receivers).astype(np.int64)
    assert N % n_cores == 0
    npc = N // n_cores                      # nodes per core
    gpc = ceil_div(npc, P)                  # groups per core
    slots = gpc * P

    deg = np.bincount(receivers, minlength=N)
    edges_of_core = [np.where(receivers // npc == c)[0] for c in range(n_cores)]

    # Bin-pack each core's nodes into gpc groups of <=128 slots, balancing edges.
    group_of_node = np.full(N, -1, np.int64)
    slot_of_node = np.full(N, -1, np.int64)
    max_group_edges = 0
    for c in range(n_cores):
        local = np.arange(npc * c, npc * (c + 1))
        order = local[np.argsort(-deg[local], kind="stable")]
        gsum = np.zeros(gpc, np.int64)
        gcnt = np.zeros(gpc, np.int64)
        for n in order:
            cand = np.where(gcnt < P)[0]
            g = cand[np.argmin(gsum[cand])]
            group_of_node[n] = g
            slot_of_node[n] = gcnt[g]
            gsum[g] += deg[n]
            gcnt[g] += 1
        max_group_edges = max(max_group_edges, int(gsum.max()))

    B = max(512, ceil_div(max_group_edges, 512) * 512)   # edge budget per group
    E_pad = gpc * B

    per_core = []
    for c in range(n_cores):
        eix = edges_of_core[c]
        g_of_e = group_of_node[receivers[eix]]
        send_idx = np.zeros(E_pad, np.int64)
        recv_idx = np.zeros(E_pad, np.int64)
        recv_slot = np.full(E_pad, 255.0, np.float32)
        edge_perm = np.full(E_pad, -1, np.int64)
        for g in range(gpc):
            ge = eix[g_of_e == g]
            assert len(ge) <= B, f"group overflow {len(ge)} > {B}"
            base = g * B
            send_idx[base:base + len(ge)] = senders[ge]
            recv_idx[base:base + len(ge)] = receivers[ge]
            recv_slot[base:base + len(ge)] = slot_of_node[receivers[ge]].astype(np.float32)
            edge_perm[base:base + len(ge)] = ge

        edgesT = np.zeros((D_EDGE, E_pad), np.float32)
        real = edge_perm >= 0
        edgesT[:, real] = np.asarray(edges, np.float32)[edge_perm[real]].T

        upd_gather = np.zeros(slots, np.int64)
        scat_idx = np.full(slots, 2_000_000, np.int64)
        local = np.arange(npc * c, npc * (c + 1))
        s_of = group_of_node[local] * P + slot_of_node[local]
        upd_gather[s_of] = local
        scat_idx[s_of] = local - npc * c

        oh2 = np.zeros((P, E_pad), np.float32)
        real_j = np.where(recv_slot < P)[0]
        oh2[recv_slot[real_j].astype(np.int64), real_j] = 1.0

        per_core.append({
            "send_idx": send_idx.astype(np.int32).reshape(-1, P).T.copy(),
            "oh2": oh2.astype(NP_BF16),
            "recv_slot": recv_slot.reshape(-1, P).T.copy(),
            "edgesT": edgesT.astype(NP_BF16),
            "upd_gather": upd_gather.astype(np.int32).reshape(-1, P).T.copy(),
            "scat_idx": scat_idx.astype(np.int32).reshape(-1, P).T.copy(),
        })

    s0 = int(group_of_node[0] * P + slot_of_node[0])
    cfg = dict(N=N, E=E, D_NODE=D_NODE, D_EDGE=D_EDGE, n_cores=n_cores,
               npc=npc, gpc=gpc, slots=slots, B=B, E_pad=E_pad, s0=s0)
    return cfg, per_core


def pack_params(params, n_layers):
    """Flatten params into {name: np.ndarray|float}. Weights are bf16 in
    matmul lhsT layout [din, dout]; biases are f32 [128, mch].

    msg0 weight rows are reordered from [edge, xs, xr] to [xs, xr, edge] so
    that 128-row K-chunks align with the rhs source tiles.
    """
    out = {}

    def put_w(name, w):
        out[name] = np.ascontiguousarray(np.asarray(w, np.float32)).astype(NP_BF16)

    def put_b(name, b):
        b = np.asarray(b, np.float32)
        dout = b.shape[0]
        mch = ceil_div(dout, P)
        bp = np.zeros((mch * P,), np.float32)
        bp[:dout] = b
        out[name] = bp.reshape(mch, P).T.copy()

    def put_mlp(prefix, mlp):
        for i, d in enumerate(mlp):
            put_w(f"{prefix}{i}_w", d["w"])
            put_b(f"{prefix}{i}_b", d["b"])

    for l in range(n_layers):
        p = params[f"layer{l}"]
        put_mlp(f"l{l}_msg", p["msg"])
        w1 = np.asarray(p["msg"][0]["w"], np.float32)
        d_in = (w1.shape[0] - 16) // 2  # D_EDGE = 16 for this model family
        d_e = w1.shape[0] - 2 * d_in
        out[f"l{l}_msg0_w"] = np.ascontiguousarray(
            np.concatenate([w1[d_e:d_e + d_in], w1[d_e + d_in:], w1[:d_e]],
                           axis=0)).astype(NP_BF16)
        put_mlp(f"l{l}_gate", p["gate"])
        put_w(f"l{l}_attn_w", p["attn"]["w"])
        out[f"l{l}_attn_b"] = float(np.asarray(p["attn"]["b"]).reshape(()))
        put_mlp(f"l{l}_upd", p["upd"])
    put_mlp("head", params["head"])
    return out


# ----------------------------------------------------------------------------
# Bass program builder
# ----------------------------------------------------------------------------

class Builder:
    def __init__(self, cfg, pp, n_layers):
        self.cfg = cfg
        self.pp = pp
        self.n_layers = n_layers
        self.MSG = pp["l0_msg2_w"].shape[1]
        self.OUT = pp[f"l{n_layers-1}_upd2_w"].shape[1]
        self.nc = bacc.Bacc("TRN2", target_bir_lowering=False, debug=False,
                            num_devices=cfg["n_cores"])
        self.input_names = []

    def inp(self, name, shape, dtype):
        h = self.nc.dram_tensor(name, list(shape), dtype, kind="ExternalInput")
        self.input_names.append(name)
        return h

    def build(self):
        cfg = self.cfg
        nc = self.nc
        gpc, E_pad = cfg["gpc"], cfg["E_pad"]
        D_EDGE, D_NODE = cfg["D_EDGE"], cfg["D_NODE"]

        nodes_in = self.inp("nodes", [cfg["N"], D_NODE], BF16)
        edgesT_in = self.inp("edgesT", [D_EDGE, E_pad], BF16)
        send_in = self.inp("send_idx", [P, E_pad // P], I32)
        oh2_in = self.inp("oh2", [P, E_pad], BF16)
        rslot_in = self.inp("recv_slot", [P, E_pad // P], F32)
        updg_in = self.inp("upd_gather", [P, gpc], I32)
        scat_in = self.inp("scat_idx", [P, gpc], I32)
        iota_in = self.inp("iota_mat", [P, P], F32)

        w_in = {}
        for name, arr in self.pp.items():
            if isinstance(arr, float):
                continue
            dt = BF16 if arr.dtype == NP_BF16 else F32
            w_in[name] = self.inp(name, list(arr.shape), dt)

        out_t = nc.dram_tensor("out", [1, 1], F32, kind="ExternalOutput")

        with tile.TileContext(nc) as tc:
            self._emit(tc, nodes_in, edgesT_in, send_in, oh2_in, rslot_in,
                       updg_in, scat_in, iota_in, w_in, out_t)
        nc.compile()
        return nc

    # ------------------------------------------------------------------
    def _emit(self, tc, nodes_in, edgesT_in, send_in, oh2_in, rslot_in,
              updg_in, scat_in, iota_in, w_in, out_t):
        cfg = self.cfg
        nc = self.nc
        gpc, B, E_pad, slots = cfg["gpc"], cfg["B"], cfg["E_pad"], cfg["slots"]
        D_EDGE, D_NODE, MSG, OUT = cfg["D_EDGE"], cfg["D_NODE"], self.MSG, self.OUT
        npc = cfg["npc"]
        n_subt = B // 512
        n_layers = self.n_layers

        import contextlib
        ctx = contextlib.ExitStack()
        with ctx:
            const_pool = ctx.enter_context(tc.tile_pool(name="const", bufs=1))
            wpool = ctx.enter_context(tc.tile_pool(name="w", bufs=1))
            sb = ctx.enter_context(tc.tile_pool(name="sb", bufs=2))
            gath = ctx.enter_context(tc.tile_pool(name="gath", bufs=3))
            sbT = ctx.enter_context(tc.tile_pool(name="sbT", bufs=1))
            xsT_pool = ctx.enter_context(tc.tile_pool(name="xsTp", bufs=2))
            xnp_pool = ctx.enter_context(tc.tile_pool(name="xnp", bufs=1))
            act = ctx.enter_context(tc.tile_pool(name="act", bufs=2))
            ps = ctx.enter_context(tc.tile_pool(name="ps", bufs=3, space="PSUM"))
            ps_t = ctx.enter_context(tc.tile_pool(name="ps_t", bufs=2, space="PSUM"))
            ps_sc = ctx.enter_context(tc.tile_pool(name="ps_sc", bufs=1, space="PSUM"))
            ps_ag = ctx.enter_context(tc.tile_pool(name="ps_ag", bufs=2, space="PSUM"))

            # ---- constants ----
            identity = const_pool.tile([P, P], BF16, tag="identity", name="identity")
            make_identity(nc, identity[:])
            iota_sb = const_pool.tile([P, P], F32, tag="iota", name="iota")
            nc.sync.dma_start(iota_sb[:], iota_in[:])
            ones_col = const_pool.tile([P, 1], BF16, tag="ones", name="ones")
            nc.vector.memset(ones_col[:], 1.0)

            # ---- kernel-resident index arrays ----
            send_sb = const_pool.tile([P, E_pad // P], I32, tag="send", name="send")
            nc.sync.dma_start(send_sb[:], send_in[:])
            rslot_sb = const_pool.tile([P, E_pad // P], F32, tag="rslot", name="rslot")
            nc.sync.dma_start(rslot_sb[:], rslot_in[:])
            updg_sb = const_pool.tile([P, gpc], I32, tag="updg", name="updg")
            nc.sync.dma_start(updg_sb[:], updg_in[:])
            scat_sb = const_pool.tile([P, gpc], I32, tag="scat", name="scat")
            nc.sync.dma_start(scat_sb[:], scat_in[:])

            # ---- DRAM scratch (raw tensors: indirect DMA needs offset-0 APs) ----
            ag_in = [nc.dram_tensor(f"ag_in{l}", [npc, MSG], BF16)
                     for l in range(n_layers - 1)]
            ag_out = [nc.dram_tensor(f"ag_out{l}", [cfg["N"], MSG], BF16,
                                     addr_space="Shared")
                      for l in range(n_layers - 1)]

            def load_w(name, tag):
                arr = self.pp[name]
                K, M = arr.shape
                tiles = []
                for k0 in range(0, K, P):
                    kk = min(P, K - k0)
                    t = wpool.tile([P, M], BF16, tag=f"{tag}_{k0 // P}",
                                   name=f"{tag}_{k0 // P}")
                    nc.sync.dma_start(t[:kk, :], w_in[name][k0:k0 + kk, :])
                    tiles.append((t, kk))
                return tiles

            def load_b(name, tag):
                arr = self.pp[name]
                t = wpool.tile([P, arr.shape[1]], F32, tag=tag, name=tag)
                nc.sync.dma_start(t[:], w_in[name][:])
                return t

            def mm_stage(rhs_tiles, w_tiles, b_tile, n_width, func, out_tag,
                         out_sb=None, out_col0=0):
                """Feature-major MLP stage: out[m][128, n_width] = func(W.T@rhs + b).

                func=None -> bias-add on DVE (no activation). out_sb writes
                into caller-provided tiles at column out_col0."""
                assert len(rhs_tiles) == len(w_tiles), (len(rhs_tiles), len(w_tiles))
                M_tot = w_tiles[0][0].shape[1]
                mch = ceil_div(M_tot, P)
                outs = []
                for m in range(mch):
                    mm0 = m * P
                    mw = min(P, M_tot - mm0)
                    pt = ps.tile([P, 512], F32, tag="ps", name="ps")
                    for ki, ent in enumerate(rhs_tiles):
                        rt, kk, col0 = ent[0], ent[1], ent[2] + (
                            ent[3] if len(ent) > 3 else 0)
                        wt, wkk = w_tiles[ki]
                        assert wkk == kk, (wkk, kk)
                        nc.tensor.matmul(
                            pt[:mw, :n_width],
                            lhsT=wt[:kk, mm0:mm0 + mw],
                            rhs=rt[:kk, col0:col0 + n_width],
                            start=(ki == 0), stop=(ki == len(rhs_tiles) - 1))
                    if out_sb is None:
                        o = act.tile([P, 512], BF16, tag=f"{out_tag}{m}",
                                     name=f"{out_tag}{m}")
                        dst = o[:mw, :n_width]
                    else:
                        o = out_sb[m]
                        dst = o[:mw, out_col0:out_col0 + n_width]
                    if func is None:
                        nc.vector.tensor_scalar(
                            dst, pt[:mw, :n_width], b_tile[:mw, m:m + 1], None,
                            ALU.add)
                    else:
                        nc.scalar.activation(dst, pt[:mw, :n_width], func,
                                             bias=b_tile[:mw, m:m + 1])
                    outs.append(o)
                return outs

            def transpose_in(src_tiles, n_rows_tot, d_feat, out_pool, out_tag,
                             out_sb=None, out_col0=0, out_tot=None):
                """Transpose row-major 128-row tiles into feature-major tiles
                [128, out_tot] (one per feat chunk of d_feat)."""
                fch = ceil_div(d_feat, P)
                if out_tot is None:
                    out_tot = n_rows_tot
                outs = []
                for f in range(fch):
                    fw = min(P, d_feat - f * P)
                    if out_sb is None:
                        o = out_pool.tile([P, out_tot], BF16, tag=f"{out_tag}{f}",
                                          name=f"{out_tag}{f}")
                    else:
                        o = out_sb[f]
                    for c0 in range(0, n_rows_tot, 512):
                        cw = min(512, n_rows_tot - c0)
                        pt = ps_t.tile([P, 512], BF16, tag="psT", name="psT")
                        for b0 in range(0, cw, P):
                            bw = min(P, cw - b0)
                            rt = src_tiles[(c0 + b0) // P]
                            nc.tensor.transpose(
                                pt[:fw, b0:b0 + bw],
                                in_=rt[:bw, f * P:f * P + fw],
                                identity=identity[:])
                        nc.vector.tensor_copy(
                            o[:fw, out_col0 + c0:out_col0 + c0 + cw],
                            pt[:fw, :cw])
                    outs.append(o)
                return outs

            def gather_rows(src_dram, idx_col_fn, n_rows, d, tag):
                """Indirect gather of n_rows (multiple of 128) rows of width d."""
                tiles = []
                for r0 in range(0, n_rows, P):
                    c = r0 // P
                    t = gath.tile([P, 256], BF16, tag=f"{tag}{c % 4}",
                                  name=f"{tag}{c % 4}")
                    nc.gpsimd.indirect_dma_start(
                        out=t[:, :d], out_offset=None, in_=src_dram,
                        in_offset=bass.IndirectOffsetOnAxis(ap=idx_col_fn(c), axis=0))
                    tiles.append(t)
                return tiles

            # ================= layers =================
            x_src = nodes_in[:]
            xnewT_last = None
            xnode_of_group = {}

            for l in range(n_layers):
                d_in = D_NODE if l == 0 else MSG
                in_fch = d_in // P

                msg_w = [load_w(f"l{l}_msg{i}_w", f"wm{i}") for i in range(3)]
                msg_b = [load_b(f"l{l}_msg{i}_b", f"bm{i}") for i in range(3)]
                gate_w = [load_w(f"l{l}_gate{i}_w", f"wg{i}") for i in range(2)]
                gate_b = [load_b(f"l{l}_gate{i}_b", f"bg{i}") for i in range(2)]
                attn_w = load_w(f"l{l}_attn_w", "wa")
                attn_b = wpool.tile([P, 1], F32, tag="ab", name="ab")
                nc.vector.memset(attn_b[:], self.pp[f"l{l}_attn_b"])
                upd_w = [load_w(f"l{l}_upd{i}_w", f"wu{i}") for i in range(3)]
                upd_b = [load_b(f"l{l}_upd{i}_b", f"bu{i}") for i in range(3)]

                aggrT = [sbT.tile([P, slots], BF16, tag=f"aggrT{f}",
                                  name=f"aggrT{f}") for f in range(MSG // P)]

                def emit_aggregation(p):
                    pa_, chunks_, g_ = p
                    for ci, (oh_, me_) in enumerate(chunks_):
                        nc.tensor.matmul(
                            pa_[:], lhsT=oh_[:], rhs=me_[:],
                            start=(ci == 0), stop=(ci == len(chunks_) - 1))
                    dn = act.tile([P, 1], F32, tag="dn", name="dn")
                    nc.vector.tensor_scalar_add(dn[:], pa_[:, MSG:MSG + 1], 1e-30)
                    rc = act.tile([P, 1], F32, tag="rc", name="rc")
                    nc.vector.reciprocal(rc[:], dn[:])
                    agg_nm = act.tile([P, MSG], BF16, tag="aggnm", name="aggnm")
                    nc.scalar.activation(agg_nm[:], pa_[:, :MSG], AF.Copy,
                                         scale=rc[:])
                    transpose_in([agg_nm], P, MSG, None, None, out_sb=aggrT,
                                 out_col0=g_ * P, out_tot=slots)

                pend = None
                for g in range(gpc):
                    pa = ps_ag.tile([P, MSG + 2], F32, tag="aggr", name="aggr")
                    # group receiver-node features (slot order = updg col g)
                    xnode = xnp_pool.tile([P, 256], BF16, tag=f"xnode{g}",
                                          name=f"xnode{g}")
                    nc.gpsimd.indirect_dma_start(
                        out=xnode[:, :d_in], out_offset=None, in_=x_src,
                        in_offset=bass.IndirectOffsetOnAxis(
                            ap=updg_sb[:, g:g + 1], axis=0))
                    xnode_of_group[g] = xnode
                    msgT_g = [act.tile([P, n_subt * 512], BF16, tag=f"msgg{f}",
                                       name=f"msgg{f}") for f in range(MSG // P)]
                    pt_s = ps_sc.tile([P, 4 * n_subt], F32, tag="sc", name="sc")
                    for s in range(n_subt):
                        t_idx = g * n_subt + s
                        col0 = t_idx * 4
                        xs_g = gather_rows(
                            x_src, lambda c: send_sb[:, col0 + c:col0 + c + 1],
                            512, d_in, "xs")
                        xsT = transpose_in(xs_g, 512, d_in, xsT_pool, "xsT")
                        oh2 = sb.tile([P, 512], BF16, tag="oh2", name="oh2")
                        nc.sync.dma_start(
                            oh2[:], oh2_in[:, t_idx * 512:(t_idx + 1) * 512])
                        # expand receiver features: xrT = xnode.T @ oh2
                        xrT = []
                        for f in range(in_fch):
                            pe_x = ps_t.tile([P, 512], F32, tag="psT", name="psT")
                            nc.tensor.matmul(
                                pe_x[:, :512], lhsT=xnode[:, f * P:(f + 1) * P],
                                rhs=oh2[:], start=True, stop=True)
                            xo = xsT_pool.tile([P, 512], BF16, tag=f"xrT{f}",
                                               name=f"xrT{f}")
                            nc.vector.tensor_copy(xo[:], pe_x[:, :512])
                            xrT.append(xo)
                        edg = sb.tile([D_EDGE, 512], BF16, tag="edg", name="edg")
                        nc.sync.dma_start(
                            edg[:], edgesT_in[:, t_idx * 512:(t_idx + 1) * 512])

                        rhs1 = ([(t, P, 0) for t in xsT] + [(t, P, 0) for t in xrT]
                                + [(edg, D_EDGE, 0)])
                        h1 = mm_stage(rhs1, msg_w[0], msg_b[0], 512, AF.Silu, "h1")
                        h2 = mm_stage([(t, P, 0) for t in h1], msg_w[1], msg_b[1],
                                      512, AF.Silu, "h2")
                        msgsT = mm_stage([(t, P, 0) for t in h2], msg_w[2], msg_b[2],
                                         512, None, "ms", out_sb=msgT_g,
                                         out_col0=s * 512)
                        g1 = mm_stage([(t, P, 0, s * 512) for t in msgT_g],
                                      gate_w[0], gate_b[0], 512, AF.Silu, "g1")
                        g2 = mm_stage([(t, P, 0) for t in g1], gate_w[1],
                                      gate_b[1], 512, None, "g2")

                        for ec in range(4):
                            nc.tensor.matmul(
                                pt_s[:, s * 4 + ec:s * 4 + ec + 1],
                                lhsT=g2[0][:, ec * P:(ec + 1) * P],
                                rhs=attn_w[0][0][:, 0:1],
                                start=True, stop=True)
                    e_t = act.tile([P, 4 * n_subt], F32, tag="e", name="e")
                    nc.scalar.activation(e_t[:], pt_s[:], AF.Exp,
                                         bias=attn_b[:, 0:1])

                    # build scaled one-hots + edge-major msgs now (DVE);
                    # the aggregation matmuls run one group later so the PE
                    # never waits on this DVE work (software pipelining).
                    chunks = []
                    for s in range(n_subt):
                        col0 = (g * n_subt + s) * 4
                        for ec in range(4):
                            pm = ps_t.tile([P, 512], BF16, tag="psT", name="psT")
                            for f in range(MSG // P):
                                nc.tensor.transpose(
                                    pm[:, f * P:(f + 1) * P],
                                    in_=msgT_g[f][:, s * 512 + ec * P:
                                                  s * 512 + (ec + 1) * P],
                                    identity=identity[:])
                            me = sb.tile([P, MSG + 2], BF16, tag=f"me{s}_{ec}",
                                         name=f"me{s}_{ec}")
                            nc.vector.tensor_copy(me[:, :MSG], pm[:, :MSG])
                            nc.vector.tensor_copy(me[:, MSG:MSG + 1], ones_col[:])
                            nc.vector.memset(me[:, MSG + 1:MSG + 2], 0.0)
                            oh = sb.tile([P, P], BF16, tag=f"oh{s}_{ec}",
                                         name=f"oh{s}_{ec}")
                            nc.vector.tensor_scalar(
                                oh[:], iota_sb[:],
                                rslot_sb[:, col0 + ec:col0 + ec + 1],
                                e_t[:, s * 4 + ec:s * 4 + ec + 1],
                                ALU.is_equal, ALU.mult)
                            chunks.append((oh, me))
                    if pend is not None:
                        emit_aggregation(pend)
                    pend = (pa, chunks, g)

                if pend is not None:
                    emit_aggregation(pend)
                    pend = None

                # ---- node update ----
                xT = transpose_in([xnode_of_group[g] for g in range(gpc)],
                                  slots, d_in, sbT, "xT")

                xnewT = [sbT.tile([P, slots], BF16, tag=f"xnT{f}", name=f"xnT{f}")
                         for f in range(OUT // P)]
                for c0 in range(0, slots, 512):
                    cw = min(512, slots - c0)
                    rhs_u = ([(t, P, c0) for t in xT] + [(t, P, c0) for t in aggrT])
                    u1 = mm_stage(rhs_u, upd_w[0], upd_b[0], cw, AF.Silu, "h1")
                    u2 = mm_stage([(t, P, 0) for t in u1], upd_w[1], upd_b[1],
                                  cw, AF.Silu, "h2")
                    u3 = mm_stage([(t, P, 0) for t in u2], upd_w[2], upd_b[2],
                                  cw, None, "u3")
                    for f in range(OUT // P):
                        nc.vector.tensor_copy(xnewT[f][:, c0:c0 + cw],
                                              u3[f][:, :cw])
                    if l > 0:
                        for f in range(OUT // P):
                            nc.scalar.activation(
                                xT[f][:, c0:c0 + cw], xT[f][:, c0:c0 + cw],
                                AF.Copy, scale=0.5)
                            nc.vector.tensor_tensor(
                                xnewT[f][:, c0:c0 + cw],
                                xnewT[f][:, c0:c0 + cw],
                                xT[f][:, c0:c0 + cw], op=ALU.add)
                    if l < n_layers - 1:
                        for g in range(c0 // P, (c0 + cw) // P):
                            pn = ps_t.tile([P, 512], BF16, tag="psT", name="psT")
                            for f in range(MSG // P):
                                nc.tensor.transpose(
                                    pn[:, f * P:(f + 1) * P],
                                    in_=xnewT[f][:, g * P:(g + 1) * P],
                                    identity=identity[:])
                            xn = sb.tile([P, MSG], BF16, tag=f"xn{g % 2}",
                                         name=f"xn{g % 2}")
                            nc.vector.tensor_copy(xn[:], pn[:, :MSG])
                            nc.gpsimd.indirect_dma_start(
                                out=ag_in[l][:],
                                out_offset=bass.IndirectOffsetOnAxis(
                                    ap=scat_sb[:, g:g + 1], axis=0),
                                in_=xn[:], in_offset=None,
                                bounds_check=npc - 1, oob_is_err=False)

                if l < n_layers - 1:
                    nc.gpsimd.collective_compute(
                        "AllGather", ALU.bypass,
                        ins=[ag_in[l][:]], outs=[ag_out[l][:]],
                        replica_groups=[list(range(cfg["n_cores"]))])
                    x_src = ag_out[l][:]
                else:
                    xnewT_last = xnewT

            # ================= head (core 0's result is the output) ==========
            s0 = cfg["s0"]
            head_w = [load_w("head0_w", "wm0"), load_w("head1_w", "wm1"),
                      load_w("head2_w", "wm2"), load_w("head3_w", "wa")]
            head_b = [load_b(f"head{i}_b", f"bm{i % 3}") for i in range(4)]
            cur = [(t, P, s0) for t in xnewT_last]
            for hi in range(3):
                M_tot = head_w[hi][0][0].shape[1]
                mch = ceil_div(M_tot, P)
                outs = []
                for m in range(mch):
                    mw = min(P, M_tot - m * P)
                    pt = ps_sc.tile([P, 4], F32, tag="sc", name="sc")
                    for ki, (rt, kk, col0) in enumerate(cur):
                        nc.tensor.matmul(
                            pt[:mw, 0:1],
                            lhsT=head_w[hi][ki][0][:kk, m * P:m * P + mw],
                            rhs=rt[:kk, col0:col0 + 1],
                            start=(ki == 0), stop=(ki == len(cur) - 1))
                    o = act.tile([P, 1], BF16, tag=f"hh{hi}_{m}", name=f"hh{hi}_{m}")
                    nc.scalar.activation(o[:mw, :], pt[:mw, 0:1], AF.Silu,
                                         bias=head_b[hi][:mw, m:m + 1])
                    outs.append((o, mw, 0))
                cur = outs
            pt = ps_sc.tile([P, 4], F32, tag="sc", name="sc")
            assert len(cur) == 1
            nc.tensor.matmul(pt[:1, 0:1],
                             lhsT=head_w[3][0][0][:, 0:1],
                             rhs=cur[0][0][:, 0:1],
                             start=True, stop=True)
            ot = act.tile([P, 1], F32, tag="outt", name="outt")
            nc.scalar.activation(ot[:1, :], pt[:1, 0:1], AF.Tanh,
                                 bias=head_b[3][:1, 0:1])
            nc.sync.dma_start(out_t[:], ot[:1, :])


# ----------------------------------------------------------------------------

def build_and_inputs(nodes, edges, senders, receivers, params, n_cores=8,
                     n_layers=4):
    nodes = np.ascontiguousarray(np.asarray(nodes, np.float32))
    edges = np.ascontiguousarray(np.asarray(edges, np.float32))
    cfg, per_core = preprocess(nodes, edges, senders, receivers, n_cores)
    pp = pack_params(params, n_layers)
    b = Builder(cfg, pp, n_layers)
    nc = b.build()

    iota_mat = np.tile(np.arange(P, dtype=np.float32), (P, 1))
    nodes_bf = nodes.astype(NP_BF16)
    in_maps = []
    for c in range(n_cores):
        m = {"nodes": nodes_bf, "iota_mat": iota_mat}
        pc = per_core[c]
        for k in ("edgesT", "send_idx", "oh2", "recv_slot",
                  "upd_gather", "scat_idx"):
            m[k] = np.ascontiguousarray(pc[k])
        for name, arr in pp.items():
            if isinstance(arr, float):
                continue
            m[name] = np.ascontiguousarray(arr)
        in_maps.append(m)
    return nc, in_maps, cfg


def golden(nodes, edges, senders, receivers, params, n_layers=4):
    """Numpy mirror of the reference model (any sizes)."""
    def apply(d, x):
        return x @ np.asarray(d["w"], np.float32) + np.asarray(d["b"], np.float32)

    def swish(x):
        return x / (1.0 + np.exp(-x))

    N = nodes.shape[0]
    x = np.asarray(nodes, np.float32)
    senders = np.asarray(senders)
    receivers = np.asarray(receivers)
    for l in range(n_layers):
        p = params[f"layer{l}"]
        h = np.concatenate([edges, x[senders], x[receivers]], axis=-1)
        for d in p["msg"][:-1]:
            h = swish(apply(d, h))
        msgs = apply(p["msg"][-1], h)
        g = swish(apply(p["gate"][0], msgs))
        g = apply(p["gate"][1], g)
        scores = apply(p["attn"], g)[:, 0]
        e = np.exp(scores)
        denom = np.zeros(N, np.float32)
        np.add.at(denom, receivers, e)
        attn = e / denom[receivers]
        aggr = np.zeros((N, msgs.shape[1]), np.float32)
        np.add.at(aggr, receivers, attn[:, None] * msgs)
        hh = np.concatenate([x, aggr], axis=-1)
        for d in p["upd"][:-1]:
            hh = swish(apply(d, hh))
        new = apply(p["upd"][-1], hh)
        if new.shape[-1] == x.shape[-1]:
            new = new + x * 0.5
        x = new
    ego = x[0:1]
    for d in params["head"][:-1]:
        ego = swish(apply(d, ego))
    return np.tanh(apply(params["head"][-1], ego))[0, 0]


# ----------------------------------------------------------------------------
# Harness entry point
# ----------------------------------------------------------------------------

N_CORES = 8
N_LAYERS = 4


def kernel(nodes, edges, senders, receivers, params):
    """Full-input entry: shards across 8 NeuronCores, runs the Bass kernel,
    returns the scalar CBF output (matches reference(**inputs))."""
    from concourse.bass_utils import run_bass_kernel_spmd

    nodes = np.asarray(nodes, np.float32)
    edges = np.asarray(edges, np.float32)
    nc, in_maps, cfg = build_and_inputs(nodes, edges, senders, receivers,
                                        params, n_cores=N_CORES,
                                        n_layers=N_LAYERS)
    res = run_bass_kernel_spmd(nc, in_maps, core_ids=list(range(N_CORES)))
    out = np.asarray(res.results[0]["out"], np.float32).reshape(())
    return out
